# revision 3
# baseline (speedup 1.0000x reference)
"""Trainium2 Bass kernel for CropUpsample (bbox-from-mask -> pad -> crop -> bilinear 256x256).

Contract: kernel(X, mask) takes the FULL inputs (X [16,32,512,512] f32,
mask [16,1,512,512] f32) and returns the FULL output [16,32,256,256] f32.

Strategy: pure data parallel over batch N across 8 NeuronCores (2 samples per
core). All data-dependent work (bbox reduction, pad arithmetic, bilinear
source indices/weights) is computed ON DEVICE in exact f32 integer arithmetic.
The crop+resize itself is expressed as two dense matmuls per (sample,channel):

    T1T[w, oy]  = sum_h X[h, w] * AyT[h, oy]      (vertical interp, X as weights)
    out[oy, ox] = sum_w T1T[w, oy] * G[w, ox]     (horizontal interp)

where AyT/G are one-hot-pair interpolation matrices built on device from the
mask-derived bbox. This is bit-equivalent to the 4-tap bilinear gather (matrix
rows have exactly two nonzeros (1-w), w) while using only dense static-shape
ops: no data-dependent DMA at all.
"""

import os
from contextlib import ExitStack

import numpy as np

N, C, H, W = 16, 32, 512, 512
HOUT, WOUT = 256, 256
NCORES = 8
NPC = N // NCORES          # samples per core
KT = H // 128              # contraction chunks (h and w)
MT = W // 128              # w chunks for stage 1 output
MO = HOUT // 128           # oy chunks for stage 2 output

F = np.float32
BIG = 1048576.0            # 2**20
EPS_FLOOR = 0.5 - 2.0 ** -11
MAGIC = 12582912.0         # 1.5 * 2**23 : uniform ULP=1 range for round-to-int


def _host_consts():
    idx_h = (np.arange(128, dtype=F)[:, None]
             + 128.0 * np.arange(KT, dtype=F)[None, :])          # [128, KT] h = p + 128 t
    idx_w = np.broadcast_to(np.arange(W, dtype=F)[None, :], (128, W)).copy()
    return {
        "c_i2p1": (2.0 * np.arange(HOUT) + 1.0).astype(F)[None, :],   # [1, 256]
        "c_iotap": np.ascontiguousarray(idx_h),                       # [128, KT]
        "c_lo_h": np.ascontiguousarray(BIG - idx_h).astype(F),        # [128, KT]
        "c_hi_h": np.ascontiguousarray(idx_h + 1.0 + BIG).astype(F),  # [128, KT]
        "c_lo_w": np.ascontiguousarray(BIG - idx_w).astype(F),        # [128, W]
        "c_hi_w": np.ascontiguousarray(idx_w + 1.0 + BIG).astype(F),  # [128, W]
        "c_sign": np.array([[-1.0, 1.0, -1.0, 1.0]], F),              # [1, 4]
        "c_dflt": np.array([[0.0, float(H), 0.0, float(W)]], F),      # [1, 4]
    }


def _floor_inplace(nc, out, in_ap, op):
    """out = floor(in_ap) for in_ap >= 0 with fractional parts that are
    multiples of 1/512 (or 1/2). 3 exact f32 adds."""
    op(out, in_ap, -EPS_FLOOR)
    op(out, out, MAGIC)
    op(out, out, -MAGIC)


def build_program():
    import concourse.mybir as mybir
    import concourse.tile as tile
    from concourse import bacc
    from concourse.bass_isa import ReduceOp
    from concourse.bass import MemorySpace

    dt = mybir.dt
    Alu = mybir.AluOpType
    Ax = mybir.AxisListType

    nc = bacc.Bacc(
        "TRN2",
        target_bir_lowering=False,
        debug=False,
        enable_asserts=False,
        num_devices=NCORES,
    )

    Xs = nc.dram_tensor("Xs", [NPC, C, H, W], dt.float32, kind="ExternalInput").ap()
    Ms = nc.dram_tensor("Ms", [NPC, 1, H, W], dt.float32, kind="ExternalInput").ap()
    Os = nc.dram_tensor("Os", [NPC, C, HOUT, WOUT], dt.float32, kind="ExternalOutput").ap()
    cdescs = {k: v for k, v in _host_consts().items()}
    cdram = {
        k: nc.dram_tensor(k, list(v.shape), dt.float32, kind="ExternalInput").ap()
        for k, v in cdescs.items()
    }

    with ExitStack() as ctx:
        tc = ctx.enter_context(tile.TileContext(nc))

        singles = ctx.enter_context(tc.tile_pool(name="singles", bufs=1))
        p_mask = ctx.enter_context(tc.tile_pool(name="mask", bufs=2))
        p_small = ctx.enter_context(tc.tile_pool(name="small", bufs=2))
        p_rows = ctx.enter_context(tc.tile_pool(name="rows", bufs=2))
        p_bc = ctx.enter_context(tc.tile_pool(name="bc", bufs=2))
        p_eq = ctx.enter_context(tc.tile_pool(name="eq", bufs=3))
        p_x = ctx.enter_context(tc.tile_pool(name="xin", bufs=4))
        p_t1 = ctx.enter_context(tc.tile_pool(name="t1", bufs=2))
        p_osb = ctx.enter_context(tc.tile_pool(name="osb", bufs=4))
        p_ps1 = ctx.enter_context(
            tc.tile_pool(name="ps1", bufs=3, space=MemorySpace.PSUM)
        )
        p_ps2 = ctx.enter_context(
            tc.tile_pool(name="ps2", bufs=3, space=MemorySpace.PSUM)
        )

        # ---- load constants into SBUF once ----
        csb = {}
        for k, v in cdescs.items():
            t = singles.tile(list(v.shape), dt.float32, tag=k)
            nc.sync.dma_start(t[:], cdram[k][:])
            csb[k] = t

        vec = nc.vector

        def ts(out, in0, s1, op0, s2=None, op1=None):
            if op1 is None:
                vec.tensor_scalar(out, in0, s1, None, op0)
            else:
                vec.tensor_scalar(out, in0, s1, s2, op0, op1)

        def build_amats(n):
            """bbox -> pad -> bilinear params -> interp matrices for sample n.
            Returns (Ay, Gx) SBUF tiles [128, KT, 256]: Ay[p,t,oy] = AyT[128t+p, oy]."""
            mk = p_mask.tile([128, KT, W], dt.float32, tag="mk")
            nc.sync.dma_start(mk[:], Ms[n, 0].rearrange("(t p) w -> p t w", p=128))

            # occupancy candidates (max-reduce forms; min via negation)
            hmax = p_small.tile([128, KT], dt.float32, tag="hmax")
            vec.tensor_reduce(hmax[:], mk[:], axis=Ax.X, op=Alu.max)
            cm = p_small.tile([128, W], dt.float32, tag="cm")
            vec.tensor_max(cm[:], mk[:, 0, :], mk[:, 1, :])
            vec.tensor_max(cm[:], cm[:], mk[:, 2, :])
            vec.tensor_max(cm[:], cm[:], mk[:, 3, :])

            vh = p_small.tile([128, KT], dt.float32, tag="vh")
            ts(vh[:], hmax[:], 0.5, Alu.is_ge)
            vw = p_small.tile([128, W], dt.float32, tag="vw")
            ts(vw[:], cm[:], 0.5, Alu.is_ge)

            R4 = p_small.tile([128, 4], dt.float32, tag="R4")
            th = p_small.tile([128, KT], dt.float32, tag="th")
            tw = p_small.tile([128, W], dt.float32, tag="tw")
            for j, cst in ((0, "c_lo_h"), (1, "c_hi_h")):
                vec.tensor_mul(th[:], vh[:], csb[cst][:])
                ts(th[:], th[:], -BIG, Alu.add)
                vec.tensor_reduce(R4[:, j : j + 1], th[:], axis=Ax.X, op=Alu.max)
            for j, cst in ((2, "c_lo_w"), (3, "c_hi_w")):
                vec.tensor_mul(tw[:], vw[:], csb[cst][:])
                ts(tw[:], tw[:], -BIG, Alu.add)
                vec.tensor_reduce(R4[:, j : j + 1], tw[:], axis=Ax.X, op=Alu.max)

            R4r = p_small.tile([128, 4], dt.float32, tag="R4r")
            nc.gpsimd.partition_all_reduce(R4r[:], R4[:], 128, ReduceOp.max)

            # scalar pipeline on partition 0: [lo_h, hi_h, lo_w, hi_w]
            bb = p_small.tile([1, 4], dt.float32, tag="bb")
            sq = p_small.tile([1, 4], dt.float32, tag="sq")
            vec.tensor_mul(bb[:], R4r[0:1, :], csb["c_sign"][:])
            vec.tensor_mul(sq[:], bb[:], bb[:])
            ts(sq[:], sq[:], (BIG / 2.0) ** 2, Alu.is_lt)          # valid mask
            vec.tensor_sub(bb[:], bb[:], csb["c_dflt"][:])
            vec.tensor_mul(bb[:], bb[:], sq[:])
            vec.tensor_add(bb[:], bb[:], csb["c_dflt"][:])         # exact bbox

            # pad to square (L == 1): grow short side, clamp at 0 / 512
            lo2 = p_small.tile([1, 2], dt.float32, tag="lo2")      # [ymin, xmin]
            hi2 = p_small.tile([1, 2], dt.float32, tag="hi2")      # [ymax, xmax]
            vec.tensor_copy(lo2[:, 0:1], bb[:, 0:1])
            vec.tensor_copy(lo2[:, 1:2], bb[:, 2:3])
            vec.tensor_copy(hi2[:, 0:1], bb[:, 1:2])
            vec.tensor_copy(hi2[:, 1:2], bb[:, 3:4])
            hwv = p_small.tile([1, 2], dt.float32, tag="hwv")      # [h, w]
            vec.tensor_sub(hwv[:], hi2[:], lo2[:])
            swp = p_small.tile([1, 2], dt.float32, tag="swp")      # [w, h]
            vec.tensor_copy(swp[:, 0:1], hwv[:, 1:2])
            vec.tensor_copy(swp[:, 1:2], hwv[:, 0:1])
            rest = p_small.tile([1, 2], dt.float32, tag="rest")
            vec.tensor_sub(rest[:], swp[:], hwv[:])
            ts(rest[:], rest[:], 0.0, Alu.max)
            half = p_small.tile([1, 2], dt.float32, tag="half")
            ts(half[:], rest[:], 0.5, Alu.mult)
            _floor_inplace(nc, half[:], half[:], lambda o, i, s: ts(o, i, s, Alu.add))
            rest1 = p_small.tile([1, 2], dt.float32, tag="rest1")
            vec.tensor_tensor(rest1[:], half[:], lo2[:], Alu.min)
            newlo = p_small.tile([1, 2], dt.float32, tag="newlo")
            vec.tensor_sub(newlo[:], lo2[:], rest1[:])
            newhi = p_small.tile([1, 2], dt.float32, tag="newhi")
            vec.tensor_sub(newhi[:], rest[:], rest1[:])            # rest2
            vec.tensor_add(newhi[:], hi2[:], newhi[:])
            ts(newhi[:], newhi[:], float(H), Alu.min)
            cl = p_small.tile([1, 2], dt.float32, tag="cl")        # [hc, wc]
            vec.tensor_sub(cl[:], newhi[:], newlo[:])
            clm1 = p_small.tile([1, 2], dt.float32, tag="clm1")
            ts(clm1[:], cl[:], -1.0, Alu.add)

            # bilinear source rows/weights per axis, then interp matrices
            mats = []
            for a in range(2):
                s = p_rows.tile([1, HOUT], dt.float32, tag="s")
                ts(s[:], csb["c_i2p1"][:], cl[:, a : a + 1], Alu.mult, -256.0, Alu.add)
                ts(s[:], s[:], 0.0, Alu.max)
                ts(s[:], s[:], 1.0 / 512.0, Alu.mult)
                i0 = p_rows.tile([1, HOUT], dt.float32, tag="i0")
                _floor_inplace(
                    nc, i0[:], s[:], lambda o, i, sc: ts(o, i, sc, Alu.add)
                )
                wgt = p_rows.tile([1, HOUT], dt.float32, tag="wgt")
                vec.tensor_sub(wgt[:], s[:], i0[:])
                i1 = p_rows.tile([1, HOUT], dt.float32, tag="i1")
                ts(i1[:], i0[:], 1.0, Alu.add, clm1[:, a : a + 1], Alu.min)
                r0 = p_rows.tile([1, HOUT], dt.float32, tag="r0")
                ts(r0[:], i0[:], newlo[:, a : a + 1], Alu.add)
                r1 = p_rows.tile([1, HOUT], dt.float32, tag="r1")
                ts(r1[:], i1[:], newlo[:, a : a + 1], Alu.add)

                r0b = p_bc.tile([128, HOUT], dt.float32, tag="r0b")
                r1b = p_bc.tile([128, HOUT], dt.float32, tag="r1b")
                wb = p_bc.tile([128, HOUT], dt.float32, tag="wb")
                nc.gpsimd.partition_broadcast(r0b[:], r0[:])
                nc.gpsimd.partition_broadcast(r1b[:], r1[:])
                nc.gpsimd.partition_broadcast(wb[:], wgt[:])

                A = singles.tile([128, KT, HOUT], dt.float32, tag=f"A{n}_{a}")
                for t in range(KT):
                    eq0 = p_eq.tile([128, HOUT], dt.float32, tag="eq0")
                    eq1 = p_eq.tile([128, HOUT], dt.float32, tag="eq1")
                    iota_t = csb["c_iotap"][:, t : t + 1]
                    ts(eq0[:], r0b[:], iota_t, Alu.is_equal)
                    ts(eq1[:], r1b[:], iota_t, Alu.is_equal)
                    vec.tensor_sub(eq1[:], eq1[:], eq0[:])
                    vec.tensor_mul(eq1[:], eq1[:], wb[:])
                    vec.tensor_add(A[:, t, :], eq1[:], eq0[:])
                mats.append(A)
            return mats

        amats = [build_amats(n) for n in range(NPC)]

        for n in range(NPC):
            Ay, Gx = amats[n]
            for c in range(C):
                xt = p_x.tile([128, KT, W], dt.float32, tag="xt")
                nc.sync.dma_start(
                    xt[:], Xs[n, c].rearrange("(t p) w -> p t w", p=128)
                )
                t1 = p_t1.tile([128, MT, HOUT], dt.float32, tag="t1")
                for m in range(MT):
                    ps = p_ps1.tile([128, HOUT], dt.float32, tag="ps")
                    for k in range(KT):
                        nc.tensor.matmul(
                            ps[:],
                            xt[:, k, 128 * m : 128 * (m + 1)],
                            Ay[:, k, :],
                            start=(k == 0),
                            stop=(k == KT - 1),
                        )
                    if m % 2 == 0:
                        nc.scalar.copy(t1[:, m, :], ps[:])
                    else:
                        vec.tensor_copy(t1[:, m, :], ps[:])
                for mo in range(MO):
                    ps2 = p_ps2.tile([128, WOUT], dt.float32, tag="ps2")
                    for k in range(MT):
                        nc.tensor.matmul(
                            ps2[:],
                            t1[:, k, 128 * mo : 128 * (mo + 1)],
                            Gx[:, k, :],
                            start=(k == 0),
                            stop=(k == MT - 1),
                        )
                    osb = p_osb.tile([128, WOUT], dt.float32, tag="osb")
                    if mo % 2 == 0:
                        nc.scalar.copy(osb[:], ps2[:])
                    else:
                        vec.tensor_copy(osb[:], ps2[:])
                    nc.sync.dma_start(
                        Os[n, c, 128 * mo : 128 * (mo + 1), :], osb[:]
                    )

    nc.compile()
    return nc


_CACHE = {}


def _get_program():
    if "nc" not in _CACHE:
        _CACHE["nc"] = build_program()
        _CACHE["consts"] = _host_consts()
    return _CACHE["nc"], _CACHE["consts"]


def _run(X, mask, trace=False):
    from concourse.bass_utils import run_bass_kernel_spmd

    nc, consts = _get_program()
    X = np.ascontiguousarray(X, dtype=np.float32)
    mask = np.ascontiguousarray(mask, dtype=np.float32)
    in_maps = []
    for core in range(NCORES):
        sl = slice(core * NPC, (core + 1) * NPC)
        m = {"Xs": X[sl], "Ms": mask[sl]}
        m.update(consts)
        in_maps.append(m)
    try:
        res = run_bass_kernel_spmd(
            nc, in_maps, core_ids=list(range(NCORES)), trace=trace
        )
    except ModuleNotFoundError:
        # axon NTFF profiling hook unavailable in this container
        res = run_bass_kernel_spmd(
            nc, in_maps, core_ids=list(range(NCORES)), trace=False
        )
    out = np.concatenate([r["Os"] for r in res.results], axis=0)
    return out, res


def kernel(X, mask):
    out, _ = _run(X, mask, trace=False)
    return out


def run_traced(X, mask):
    out, res = _run(X, mask, trace=True)
    return out, res


# revision 6
# speedup vs baseline: 27041.3773x; 27041.3773x over previous
"""Trainium2 Bass kernel for CropUpsample (bbox-from-mask -> pad -> crop -> bilinear 256x256).

Contract: kernel(X, mask) takes the FULL inputs (X [16,32,512,512] f32,
mask [16,1,512,512] f32) and returns the FULL output [16,32,256,256] f32.

Strategy: pure data parallel over batch N across 8 NeuronCores (2 samples per
core). All data-dependent work (bbox reduction, pad arithmetic, bilinear
source indices/weights) is computed ON DEVICE in exact f32 integer arithmetic.
The crop+resize itself is expressed as two dense matmuls per (sample,channel):

    T1T[w, oy]  = sum_h X[h, w] * AyT[h, oy]      (vertical interp, X as weights)
    out[oy, ox] = sum_w T1T[w, oy] * G[w, ox]     (horizontal interp)

where AyT/G are one-hot-pair interpolation matrices built on device from the
mask-derived bbox. This is bit-equivalent to the 4-tap bilinear gather (matrix
rows have exactly two nonzeros (1-w), w) while using only dense static-shape
ops: no data-dependent DMA at all.
"""

import os
from contextlib import ExitStack

import numpy as np

N, C, H, W = 16, 32, 512, 512
HOUT, WOUT = 256, 256
NCORES = 8
NPC = N // NCORES          # samples per core
KT = H // 128              # contraction chunks (h and w)
MT = W // 128              # w chunks for stage 1 output
MO = HOUT // 128           # oy chunks for stage 2 output

F = np.float32
BIG = 1048576.0            # 2**20
EPS_FLOOR = 0.5 - 2.0 ** -11
MAGIC = 12582912.0         # 1.5 * 2**23 : uniform ULP=1 range for round-to-int


def _host_consts():
    idx_h = (np.arange(128, dtype=F)[:, None]
             + 128.0 * np.arange(KT, dtype=F)[None, :])          # [128, KT] h = p + 128 t
    idx_w = np.broadcast_to(np.arange(W, dtype=F)[None, :], (128, W)).copy()
    return {
        "c_i2p1": (2.0 * np.arange(HOUT) + 1.0).astype(F)[None, :],   # [1, 256]
        "c_iotap": np.ascontiguousarray(idx_h),                       # [128, KT]
        "c_lo_h": np.ascontiguousarray(BIG - idx_h).astype(F),        # [128, KT]
        "c_hi_h": np.ascontiguousarray(idx_h + 1.0 + BIG).astype(F),  # [128, KT]
        "c_lo_w": np.ascontiguousarray(BIG - idx_w).astype(F),        # [128, W]
        "c_hi_w": np.ascontiguousarray(idx_w + 1.0 + BIG).astype(F),  # [128, W]
        "c_sign": np.array([[-1.0, 1.0, -1.0, 1.0]], F),              # [1, 4]
        "c_dflt": np.array([[0.0, float(H), 0.0, float(W)]], F),      # [1, 4]
    }


def _floor_inplace(nc, out, in_ap, op):
    """out = floor(in_ap) for in_ap >= 0 with fractional parts that are
    multiples of 1/512 (or 1/2). 3 exact f32 adds."""
    op(out, in_ap, -EPS_FLOOR)
    op(out, out, MAGIC)
    op(out, out, -MAGIC)


def build_program(repeat=1, win=W):
    import concourse.mybir as mybir
    import concourse.tile as tile
    from concourse import bacc
    from concourse.bass_isa import ReduceOp
    from concourse.bass import MemorySpace

    import concourse.bass as bass

    dt = mybir.dt
    Alu = mybir.AluOpType
    Ax = mybir.AxisListType
    kt = win // 128            # chunks in the (cropped) window

    nc = bacc.Bacc(
        "TRN2",
        target_bir_lowering=False,
        debug=False,
        enable_asserts=False,
        num_devices=NCORES,
    )

    Xs = nc.dram_tensor("Xs", [NPC, C, H, W], dt.float32, kind="ExternalInput").ap()
    Ms = nc.dram_tensor("Ms", [NPC, 1, H, W], dt.float32, kind="ExternalInput").ap()
    Os = nc.dram_tensor("Os", [NPC, C, HOUT, WOUT], dt.float32, kind="ExternalOutput").ap()
    cdescs = {k: v for k, v in _host_consts().items()}
    cdram = {
        k: nc.dram_tensor(k, list(v.shape), dt.float32, kind="ExternalInput").ap()
        for k, v in cdescs.items()
    }

    with ExitStack() as ctx:
        tc = ctx.enter_context(tile.TileContext(nc))

        singles = ctx.enter_context(tc.tile_pool(name="singles", bufs=1))
        p_mask = ctx.enter_context(tc.tile_pool(name="mask", bufs=2))
        p_small = ctx.enter_context(tc.tile_pool(name="small", bufs=2))
        p_rows = ctx.enter_context(tc.tile_pool(name="rows", bufs=2))
        p_bc = ctx.enter_context(tc.tile_pool(name="bc", bufs=2))
        p_eq = ctx.enter_context(tc.tile_pool(name="eq", bufs=3))
        p_x = ctx.enter_context(tc.tile_pool(name="xin", bufs=4))
        p_t1 = ctx.enter_context(tc.tile_pool(name="t1", bufs=2))
        p_osb = ctx.enter_context(tc.tile_pool(name="osb", bufs=4))
        p_ps1 = ctx.enter_context(
            tc.tile_pool(name="ps1", bufs=3, space=MemorySpace.PSUM)
        )
        p_ps2 = ctx.enter_context(
            tc.tile_pool(name="ps2", bufs=3, space=MemorySpace.PSUM)
        )

        # ---- load constants into SBUF once ----
        csb = {}
        for k, v in cdescs.items():
            t = singles.tile(list(v.shape), dt.float32, tag=k)
            nc.sync.dma_start(t[:], cdram[k][:])
            csb[k] = t

        vec = nc.vector

        def ts(out, in0, s1, op0, s2=None, op1=None):
            if op1 is None:
                vec.tensor_scalar(out, in0, s1, None, op0)
            else:
                vec.tensor_scalar(out, in0, s1, s2, op0, op1)

        def build_amats(n):
            """bbox -> pad -> bilinear params -> interp matrices for sample n.
            Returns (Ay, Gx) SBUF tiles [128, KT, 256]: Ay[p,t,oy] = AyT[128t+p, oy]."""
            mk = p_mask.tile([128, KT, W], dt.float32, tag="mk")
            nc.sync.dma_start(mk[:], Ms[n, 0].rearrange("(t p) w -> p t w", p=128))

            # occupancy candidates (max-reduce forms; min via negation)
            hmax = p_small.tile([128, KT], dt.float32, tag="hmax")
            vec.tensor_reduce(hmax[:], mk[:], axis=Ax.X, op=Alu.max)
            cm = p_small.tile([128, W], dt.float32, tag="cm")
            vec.tensor_max(cm[:], mk[:, 0, :], mk[:, 1, :])
            vec.tensor_max(cm[:], cm[:], mk[:, 2, :])
            vec.tensor_max(cm[:], cm[:], mk[:, 3, :])

            vh = p_small.tile([128, KT], dt.float32, tag="vh")
            ts(vh[:], hmax[:], 0.5, Alu.is_ge)
            vw = p_small.tile([128, W], dt.float32, tag="vw")
            ts(vw[:], cm[:], 0.5, Alu.is_ge)

            R4 = p_small.tile([128, 4], dt.float32, tag="R4")
            th = p_small.tile([128, KT], dt.float32, tag="th")
            tw = p_small.tile([128, W], dt.float32, tag="tw")
            for j, cst in ((0, "c_lo_h"), (1, "c_hi_h")):
                vec.tensor_mul(th[:], vh[:], csb[cst][:])
                ts(th[:], th[:], -BIG, Alu.add)
                vec.tensor_reduce(R4[:, j : j + 1], th[:], axis=Ax.X, op=Alu.max)
            for j, cst in ((2, "c_lo_w"), (3, "c_hi_w")):
                vec.tensor_mul(tw[:], vw[:], csb[cst][:])
                ts(tw[:], tw[:], -BIG, Alu.add)
                vec.tensor_reduce(R4[:, j : j + 1], tw[:], axis=Ax.X, op=Alu.max)

            R4r = p_small.tile([128, 4], dt.float32, tag="R4r")
            nc.gpsimd.partition_all_reduce(R4r[:], R4[:], 128, ReduceOp.max)

            # scalar pipeline on partition 0: [lo_h, hi_h, lo_w, hi_w]
            bb = p_small.tile([1, 4], dt.float32, tag="bb")
            sq = p_small.tile([1, 4], dt.float32, tag="sq")
            vec.tensor_mul(bb[:], R4r[0:1, :], csb["c_sign"][:])
            vec.tensor_mul(sq[:], bb[:], bb[:])
            ts(sq[:], sq[:], (BIG / 2.0) ** 2, Alu.is_lt)          # valid mask
            vec.tensor_sub(bb[:], bb[:], csb["c_dflt"][:])
            vec.tensor_mul(bb[:], bb[:], sq[:])
            vec.tensor_add(bb[:], bb[:], csb["c_dflt"][:])         # exact bbox

            # pad to square (L == 1): grow short side, clamp at 0 / 512
            lo2 = p_small.tile([1, 2], dt.float32, tag="lo2")      # [ymin, xmin]
            hi2 = p_small.tile([1, 2], dt.float32, tag="hi2")      # [ymax, xmax]
            vec.tensor_copy(lo2[:, 0:1], bb[:, 0:1])
            vec.tensor_copy(lo2[:, 1:2], bb[:, 2:3])
            vec.tensor_copy(hi2[:, 0:1], bb[:, 1:2])
            vec.tensor_copy(hi2[:, 1:2], bb[:, 3:4])
            hwv = p_small.tile([1, 2], dt.float32, tag="hwv")      # [h, w]
            vec.tensor_sub(hwv[:], hi2[:], lo2[:])
            swp = p_small.tile([1, 2], dt.float32, tag="swp")      # [w, h]
            vec.tensor_copy(swp[:, 0:1], hwv[:, 1:2])
            vec.tensor_copy(swp[:, 1:2], hwv[:, 0:1])
            rest = p_small.tile([1, 2], dt.float32, tag="rest")
            vec.tensor_sub(rest[:], swp[:], hwv[:])
            ts(rest[:], rest[:], 0.0, Alu.max)
            half = p_small.tile([1, 2], dt.float32, tag="half")
            ts(half[:], rest[:], 0.5, Alu.mult)
            _floor_inplace(nc, half[:], half[:], lambda o, i, s: ts(o, i, s, Alu.add))
            rest1 = p_small.tile([1, 2], dt.float32, tag="rest1")
            vec.tensor_tensor(rest1[:], half[:], lo2[:], Alu.min)
            newlo = p_small.tile([1, 2], dt.float32, tag="newlo")
            vec.tensor_sub(newlo[:], lo2[:], rest1[:])
            newhi = p_small.tile([1, 2], dt.float32, tag="newhi")
            vec.tensor_sub(newhi[:], rest[:], rest1[:])            # rest2
            vec.tensor_add(newhi[:], hi2[:], newhi[:])
            ts(newhi[:], newhi[:], float(H), Alu.min)
            cl = p_small.tile([1, 2], dt.float32, tag="cl")        # [hc, wc]
            vec.tensor_sub(cl[:], newhi[:], newlo[:])
            clm1 = p_small.tile([1, 2], dt.float32, tag="clm1")
            ts(clm1[:], cl[:], -1.0, Alu.add)

            if win < W:
                # static-size dynamic-offset read window [ws, ws+win) per axis
                ws = p_small.tile([1, 2], dt.float32, tag="ws")
                ts(ws[:], newlo[:], float(H - win), Alu.min)
                nlr = p_small.tile([1, 2], dt.float32, tag="nlr")
                vec.tensor_sub(nlr[:], newlo[:], ws[:])        # window-rel lo
                ws_i = p_small.tile([1, 2], dt.int32, tag="ws_i")
                vec.tensor_copy(ws_i[:], ws[:])
                offs = [
                    nc.values_load(
                        ws_i[0:1, a : a + 1], min_val=0, max_val=H - win
                    )
                    for a in range(2)
                ]
            else:
                nlr = newlo
                offs = None

            # bilinear source rows/weights per axis, then interp matrices
            mats = []
            for a in range(2):
                s = p_rows.tile([1, HOUT], dt.float32, tag="s")
                ts(s[:], csb["c_i2p1"][:], cl[:, a : a + 1], Alu.mult, -256.0, Alu.add)
                ts(s[:], s[:], 0.0, Alu.max)
                ts(s[:], s[:], 1.0 / 512.0, Alu.mult)
                i0 = p_rows.tile([1, HOUT], dt.float32, tag="i0")
                _floor_inplace(
                    nc, i0[:], s[:], lambda o, i, sc: ts(o, i, sc, Alu.add)
                )
                wgt = p_rows.tile([1, HOUT], dt.float32, tag="wgt")
                vec.tensor_sub(wgt[:], s[:], i0[:])
                i1 = p_rows.tile([1, HOUT], dt.float32, tag="i1")
                ts(i1[:], i0[:], 1.0, Alu.add, clm1[:, a : a + 1], Alu.min)
                r0 = p_rows.tile([1, HOUT], dt.float32, tag="r0")
                ts(r0[:], i0[:], nlr[:, a : a + 1], Alu.add)
                r1 = p_rows.tile([1, HOUT], dt.float32, tag="r1")
                ts(r1[:], i1[:], nlr[:, a : a + 1], Alu.add)

                r0b = p_bc.tile([128, HOUT], dt.float32, tag="r0b")
                r1b = p_bc.tile([128, HOUT], dt.float32, tag="r1b")
                wb = p_bc.tile([128, HOUT], dt.float32, tag="wb")
                nc.gpsimd.partition_broadcast(r0b[:], r0[:])
                nc.gpsimd.partition_broadcast(r1b[:], r1[:])
                nc.gpsimd.partition_broadcast(wb[:], wgt[:])

                A = singles.tile([128, kt, HOUT], dt.float32, tag=f"A{n}_{a}")
                for t in range(kt):
                    eq0 = p_eq.tile([128, HOUT], dt.float32, tag="eq0")
                    eq1 = p_eq.tile([128, HOUT], dt.float32, tag="eq1")
                    iota_t = csb["c_iotap"][:, t : t + 1]
                    ts(eq0[:], r0b[:], iota_t, Alu.is_equal)
                    ts(eq1[:], r1b[:], iota_t, Alu.is_equal)
                    vec.tensor_sub(eq1[:], eq1[:], eq0[:])
                    vec.tensor_mul(eq1[:], eq1[:], wb[:])
                    vec.tensor_add(A[:, t, :], eq1[:], eq0[:])
                mats.append(A)
            return mats, offs

        def emit_all():
            amats = [build_amats(n) for n in range(NPC)]
            for n in range(NPC):
                emit_channels(n, amats[n])

        def emit_channels(n, built):
            (Ay, Gx), offs = built
            for c in range(C):
                xt = p_x.tile([128, kt, win], dt.float32, tag="xt")
                if offs is None:
                    src_ap = Xs[n, c].rearrange("(t p) w -> p t w", p=128)
                else:
                    src_ap = Xs[n, c][
                        bass.ds(offs[0], win), bass.ds(offs[1], win)
                    ].rearrange("(t p) w -> p t w", p=128)
                nc.sync.dma_start(xt[:], src_ap)
                t1 = p_t1.tile([128, kt, HOUT], dt.float32, tag="t1")
                for m in range(kt):
                    ps = p_ps1.tile([128, HOUT], dt.float32, tag="ps")
                    for k in range(kt):
                        nc.tensor.matmul(
                            ps[:],
                            xt[:, k, 128 * m : 128 * (m + 1)],
                            Ay[:, k, :],
                            start=(k == 0),
                            stop=(k == kt - 1),
                        )
                    if m % 2 == 0:
                        nc.scalar.copy(t1[:, m, :], ps[:])
                    else:
                        vec.tensor_copy(t1[:, m, :], ps[:])
                for mo in range(MO):
                    ps2 = p_ps2.tile([128, WOUT], dt.float32, tag="ps2")
                    for k in range(kt):
                        nc.tensor.matmul(
                            ps2[:],
                            t1[:, k, 128 * mo : 128 * (mo + 1)],
                            Gx[:, k, :],
                            start=(k == 0),
                            stop=(k == kt - 1),
                        )
                    osb = p_osb.tile([128, WOUT], dt.float32, tag="osb")
                    if mo % 2 == 0:
                        nc.scalar.copy(osb[:], ps2[:])
                    else:
                        vec.tensor_copy(osb[:], ps2[:])
                    nc.sync.dma_start(
                        Os[n, c, 128 * mo : 128 * (mo + 1), :], osb[:]
                    )

        if repeat == 1:
            emit_all()
        else:
            with tc.For_i(0, repeat, 1):
                emit_all()

    nc.compile()
    return nc


_CACHE = {}


def _get_program():
    if "nc" not in _CACHE:
        _CACHE["nc"] = build_program()
        _CACHE["consts"] = _host_consts()
    return _CACHE["nc"], _CACHE["consts"]


def _run(X, mask, trace=False):
    from concourse.bass_utils import run_bass_kernel_spmd

    nc, consts = _get_program()
    X = np.ascontiguousarray(X, dtype=np.float32)
    mask = np.ascontiguousarray(mask, dtype=np.float32)
    in_maps = []
    for core in range(NCORES):
        sl = slice(core * NPC, (core + 1) * NPC)
        m = {"Xs": X[sl], "Ms": mask[sl]}
        m.update(consts)
        in_maps.append(m)
    try:
        res = run_bass_kernel_spmd(
            nc, in_maps, core_ids=list(range(NCORES)), trace=trace
        )
    except ModuleNotFoundError:
        # axon NTFF profiling hook unavailable in this container
        res = run_bass_kernel_spmd(
            nc, in_maps, core_ids=list(range(NCORES)), trace=False
        )
    out = np.concatenate([r["Os"] for r in res.results], axis=0)
    return out, res


def kernel(X, mask):
    out, _ = _run(X, mask, trace=False)
    return out


def run_traced(X, mask):
    out, res = _run(X, mask, trace=True)
    return out, res
